# revision 14
# baseline (speedup 1.0000x reference)
"""Trainium2 Bass kernel: 4-layer pose-temporal transformer encoder.

kernel(**inputs) takes FULL unsharded fp32 inputs, returns FULL (16,512,1024)
fp32 output.  Data-parallel over batch across 8 NeuronCores (2 batch elements
per core, no collectives); bf16 matmuls with fp32 PSUM accumulation.

Per-core layout: feature-major residual stream x[E(part), tokens] fp32, updated
in place.  q/k/o/mlp weights stream as stationary lhsT in natural [K, N] layout;
the v projection uses h as lhsT so v lands token-major, which is exactly the
lhsT the A@V matmul needs.  Scores are token-major [tq(part), tk(free)];
softmax runs along the free axis with the exp's accum_out giving the
denominator; P is transposed tile-wise by DMA-transpose (bf16) to feed A@V.
The relative-position bias interpolation collapses to a constant-weight blend
of adjacent bias-table rows, so the full (L,H,T,T) Toeplitz bias is
precomputed on the host in bf16 and streamed in.
"""

import numpy as np
import ml_dtypes
from contextlib import ExitStack

import concourse.bass as bass
import concourse.tile as tile
from concourse import bacc, mybir
from concourse.bass_utils import run_bass_kernel_spmd

F32 = mybir.dt.float32
BF16 = mybir.dt.bfloat16
AF = mybir.ActivationFunctionType
ALU = mybir.AluOpType
P = 128

FULL = dict(BL=2, T=512, E=1024, H=16, FF=4096, L=4)
N_CORES = 8
EPS = 1e-5
MAX_OFFSET = 0.5


def build_nc(cfg, flags=frozenset()):
    BL, T, E, H, FF, L = cfg["BL"], cfg["T"], cfg["E"], cfg["H"], cfg["FF"], cfg["L"]
    HD = E // H
    EO = E // P
    FO = FF // P
    TOK = BL * T
    CH = min(512, T)
    NCH = TOK // CH
    TQ = T // P
    HPT = max(1, P // HD)
    WS = min(512, E)          # weight strip width

    nc = bacc.Bacc(None, target_bir_lowering=False, debug=False)

    x_d = nc.declare_dram_parameter("x_fm", [E, TOK], F32, False)
    wq_d = nc.declare_dram_parameter("wq", [L, E, E], BF16, False)
    wk_d = nc.declare_dram_parameter("wk", [L, E, E], BF16, False)
    wv_d = nc.declare_dram_parameter("wv", [L, E, E], BF16, False)
    wo_d = nc.declare_dram_parameter("wo", [L, E, E], BF16, False)
    w1_d = nc.declare_dram_parameter("w1", [L, E, FF], BF16, False)
    w2_d = nc.declare_dram_parameter("w2", [L, FF, E], BF16, False)
    bm_d = nc.declare_dram_parameter("biasmat", [L, H, T, T], BF16, False)
    extra = {}
    for nm, shp in [("bq", [L, E]), ("bk", [L, E]), ("bv", [L, E]),
                    ("bo", [L, E]), ("b1", [L, FF]), ("b2", [L, E]),
                    ("ln1_g", [L, E]), ("ln1_b", [L, E]),
                    ("ln2_g", [L, E]), ("ln2_b", [L, E])]:
        key = nm.split("_")[0] if nm.startswith("ln") else nm
        if key in flags:
            extra[nm] = nc.declare_dram_parameter(nm, shp, F32, False)
    out_d = nc.declare_dram_parameter("out_fm", [E, TOK], F32, True)

    with tile.TileContext(nc) as tc, ExitStack() as ctx:
        const = ctx.enter_context(tc.tile_pool(name="const", bufs=1))
        resid = ctx.enter_context(tc.tile_pool(name="resid", bufs=1))
        hpool = ctx.enter_context(tc.tile_pool(name="hpool", bufs=2))
        qpool = ctx.enter_context(tc.tile_pool(name="qpool", bufs=1))
        kpool = ctx.enter_context(tc.tile_pool(name="kpool", bufs=1))
        vpool = ctx.enter_context(tc.tile_pool(name="vpool", bufs=1))
        hidpool = ctx.enter_context(tc.tile_pool(name="hidpool", bufs=1))
        wpool = ctx.enter_context(tc.tile_pool(name="wpool", bufs=3))
        lnpool = ctx.enter_context(tc.tile_pool(name="lnpool", bufs=2))
        rowpool = ctx.enter_context(tc.tile_pool(name="rowpool", bufs=1))
        colpool = ctx.enter_context(tc.tile_pool(name="colpool", bufs=3))
        ppool = ctx.enter_context(tc.tile_pool(name="ppool", bufs=3))
        ptpool = ctx.enter_context(tc.tile_pool(name="ptpool", bufs=TQ + 2))
        bpool = ctx.enter_context(tc.tile_pool(name="bpool", bufs=TQ + 2))
        ps = ctx.enter_context(tc.tile_pool(name="ps", bufs=8, space="PSUM"))

        def psum(pdim, fdim, name):
            return ps.tile([pdim, fdim], F32, name=name, tag="psb")

        ones_col = const.tile([P, 1], BF16)
        nc.vector.memset(ones_col, 1.0)
        ones_row = const.tile([1, P], F32)
        nc.vector.memset(ones_row, 1.0)
        zero_col = const.tile([P, 1], F32)
        nc.vector.memset(zero_col, 0.0)
        eps_c = const.tile([1, 1], F32)
        nc.vector.memset(eps_c, EPS)

        def load_param_cols(dram_row, n_tiles, nm):
            t = const.tile([P, n_tiles], F32, name=nm, tag=nm)
            nc.sync.dma_start(out=t, in_=dram_row.rearrange("(o p) -> p o", p=P))
            return t

        params = {}
        for l in range(L):
            for nm in ("bq", "bk", "bo", "b1", "b2"):
                if nm in extra:
                    n_t = FO if nm == "b1" else EO
                    params[(nm, l)] = load_param_cols(extra[nm][l], n_t, f"{nm}{l}")
            for nm in ("ln1_g", "ln1_b", "ln2_g", "ln2_b"):
                if nm in extra:
                    params[(nm, l)] = load_param_cols(extra[nm][l], EO, f"{nm}{l}")

        x_sb = resid.tile([P, EO, TOK], F32)
        for eo in range(EO):
            nc.sync.dma_start(out=x_sb[:, eo, :], in_=x_d[eo * P:(eo + 1) * P, :])

        def layernorm(g, b):
            """LN of x_sb (feature-major, partition reduce); bf16 out."""
            out = hpool.tile([P, EO, TOK], BF16, name="hs", tag="hs")
            for c in range(NCH):
                csl = bass.ts(c, CH)
                ssum = psum(1, CH, "ssum")
                ssq = psum(1, CH, "ssq")
                for eo in range(EO):
                    xbc = lnpool.tile([P, CH], BF16, name="xbc", tag="xbc")
                    nc.vector.tensor_copy(out=xbc, in_=x_sb[:, eo, csl])
                    sqc = lnpool.tile([P, CH], BF16, name="sqc", tag="sqc")
                    nc.vector.tensor_mul(out=sqc, in0=xbc, in1=xbc)
                    nc.tensor.matmul(ssum, ones_col, xbc,
                                     start=(eo == 0), stop=(eo == EO - 1))
                    nc.tensor.matmul(ssq, ones_col, sqc,
                                     start=(eo == 0), stop=(eo == EO - 1))
                m = rowpool.tile([1, CH], F32, name="m", tag="m")
                va = rowpool.tile([1, CH], F32, name="va", tag="va")
                msq = rowpool.tile([1, CH], F32, name="msq", tag="msq")
                rstd = rowpool.tile([1, CH], F32, name="rstd", tag="rstd")
                crow = rowpool.tile([1, CH], F32, name="crow", tag="crow")
                nc.vector.tensor_scalar_mul(m, ssum, 1.0 / E)
                nc.vector.tensor_scalar_mul(va, ssq, 1.0 / E)
                nc.vector.tensor_mul(out=msq, in0=m, in1=m)
                nc.vector.tensor_sub(out=va, in0=va, in1=msq)
                nc.scalar.activation(out=va, in_=va, func=AF.Sqrt, bias=eps_c)
                nc.vector.reciprocal(out=rstd, in_=va)
                nc.vector.tensor_mul(out=crow, in0=m, in1=rstd)
                nc.vector.tensor_scalar_mul(crow, crow, -1.0)
                a_ps = psum(P, CH, "a_ps")
                nc.tensor.matmul(a_ps, ones_row, rstd, start=True, stop=True)
                c_ps = psum(P, CH, "c_ps")
                nc.tensor.matmul(c_ps, ones_row, crow, start=True, stop=True)
                for eo in range(EO):
                    t1 = lnpool.tile([P, CH], F32, name="lnt1", tag="lnt1")
                    nc.vector.tensor_mul(out=t1, in0=x_sb[:, eo, csl], in1=a_ps)
                    if g is None:
                        nc.vector.tensor_add(out=out[:, eo, csl], in0=t1, in1=c_ps)
                    else:
                        nc.vector.tensor_add(out=t1, in0=t1, in1=c_ps)
                        nc.vector.tensor_scalar(
                            out=out[:, eo, csl], in0=t1,
                            scalar1=g[:, eo:eo + 1], scalar2=b[:, eo:eo + 1],
                            op0=ALU.mult, op1=ALU.add)
            return out

        def load_strip(w2d, r0, rn, c0, cn, nm):
            """dram [rows, cols] slice -> sbuf [P, rn//P, cn]."""
            t = wpool.tile([P, rn // P, cn], BF16, name=nm, tag="w")
            for ko in range(rn // P):
                nc.sync.dma_start(
                    out=t[:, ko, :],
                    in_=w2d[r0 + ko * P: r0 + (ko + 1) * P, c0:c0 + cn])
            return t

        def proj_fm(rhs_sb, w_l, evict):
            for nh in range(E // WS):
                wt = load_strip(w_l, 0, E, nh * WS, WS, "wproj")
                for ni in range(WS // P):
                    no = nh * (WS // P) + ni
                    pss = [psum(P, CH, "pss") for _ in range(NCH)]
                    for ko in range(EO):
                        for c in range(NCH):
                            nc.tensor.matmul(
                                pss[c], wt[:, ko, ni * P:(ni + 1) * P],
                                rhs_sb[:, ko, bass.ts(c, CH)],
                                start=(ko == 0), stop=(ko == EO - 1))
                    for c in range(NCH):
                        evict(pss[c], no, c)

        def act_evict(dst, bias_tile=None):
            def f(pst, no, c):
                if bias_tile is None:
                    nc.scalar.copy(out=dst[:, no, bass.ts(c, CH)], in_=pst)
                else:
                    nc.scalar.activation(out=dst[:, no, bass.ts(c, CH)], in_=pst,
                                         func=AF.Identity,
                                         bias=bias_tile[:, no:no + 1])
            return f

        for l in range(L):
            h_sb = layernorm(params.get(("ln1_g", l)), params.get(("ln1_b", l)))

            q_sb = qpool.tile([P, EO, TOK], BF16)
            proj_fm(h_sb, wq_d[l], act_evict(q_sb, params.get(("bq", l))))
            k_sb = kpool.tile([P, EO, TOK], BF16)
            proj_fm(h_sb, wk_d[l], act_evict(k_sb, params.get(("bk", l))))

            # v: token-major (h as stationary lhsT, wv streams as rhs)
            v_sb = vpool.tile([P, TOK // P, E], BF16)
            bvb = None
            if "bv" in extra:
                bvrow = colpool.tile([1, E], F32, name="bvrow", tag="bvrow")
                nc.sync.dma_start(out=bvrow, in_=extra["bv"][l].rearrange("e -> 1 e"))
                bvb = colpool.tile([P, E], F32, name="bvb", tag="bvb")
                for j in range(E // CH):
                    bp = psum(P, CH, "bvps")
                    nc.tensor.matmul(bp, ones_row, bvrow[:, bass.ts(j, CH)],
                                     start=True, stop=True)
                    nc.scalar.copy(out=bvb[:, bass.ts(j, CH)], in_=bp)
            wvs = [load_strip(wv_d[l], 0, E, j * WS, WS, "wproj")
                   for j in range(E // WS)]
            for to in range(TOK // P):
                pss = [psum(P, WS, "pss") for _ in range(E // WS)]
                for ko in range(EO):
                    for j in range(E // WS):
                        nc.tensor.matmul(
                            pss[j], h_sb[:, ko, to * P:(to + 1) * P],
                            wvs[j][:, ko, :],
                            start=(ko == 0), stop=(ko == EO - 1))
                for j in range(E // WS):
                    if bvb is None:
                        nc.scalar.copy(out=v_sb[:, to, bass.ts(j, WS)], in_=pss[j])
                    else:
                        nc.vector.tensor_add(out=v_sb[:, to, bass.ts(j, WS)],
                                             in0=pss[j], in1=bvb[:, bass.ts(j, WS)])

            # attention; bias tiles shared across the batch dim
            ao_sb = hpool.tile([P, EO, TOK], BF16, name="hs", tag="hs")
            for h in range(H):
                po = (h % HPT) * HD
                eo_h = h // HPT
                btiles = []
                for tq in range(TQ):
                    bt = bpool.tile([P, T], BF16, name="btile", tag="btile")
                    nc.sync.dma_start(out=bt, in_=bm_d[l, h, tq * P:(tq + 1) * P, :])
                    btiles.append(bt)
                for b in range(BL):
                    pts = [ptpool.tile([P, T], BF16, name="pts", tag="pts")
                           for _ in range(TQ)]
                    for tq in range(TQ):
                        sps = psum(P, T, "sps")
                        nc.tensor.matmul(
                            sps,
                            q_sb[po:po + HD, eo_h,
                                 b * T + tq * P: b * T + (tq + 1) * P],
                            k_sb[po:po + HD, eo_h, b * T: (b + 1) * T],
                            start=True, stop=True)
                        nc.vector.tensor_add(out=sps, in0=sps, in1=btiles[tq])
                        pt_ = ppool.tile([P, T], BF16, name="pt_", tag="pt_")
                        esums = rowpool.tile([P, 2], F32, name="esums",
                                             tag="esums", bufs=4)
                        nc.scalar.activation(out=pt_, in_=sps, func=AF.Exp,
                                             bias=zero_col,
                                             accum_out=esums[:, 0:1])
                        nc.vector.reciprocal(out=esums[:, 1:2], in_=esums[:, 0:1])
                        nc.vector.tensor_scalar_mul(pt_, pt_, esums[:, 1:2])
                        for tk in range(TQ):
                            nc.sync.dma_start_transpose(
                                out=pts[tk][:, tq * P:(tq + 1) * P],
                                in_=pt_[:, tk * P:(tk + 1) * P])
                    ops = psum(HD, T, "ops")
                    for tk in range(TQ):
                        nc.tensor.matmul(
                            ops, v_sb[:, b * TQ + tk, h * HD:(h + 1) * HD], pts[tk],
                            start=(tk == 0), stop=(tk == TQ - 1))
                    nc.scalar.copy(
                        out=ao_sb[po:po + HD, eo_h, b * T:(b + 1) * T], in_=ops)

            # out projection + residual (in place)
            bo_t = params.get(("bo", l))

            def o_evict(pst, no, c):
                csl = bass.ts(c, CH)
                if bo_t is None:
                    nc.vector.tensor_add(out=x_sb[:, no, csl], in0=pst,
                                         in1=x_sb[:, no, csl])
                else:
                    nc.vector.scalar_tensor_tensor(
                        out=x_sb[:, no, csl], in0=pst,
                        scalar=bo_t[:, no:no + 1], in1=x_sb[:, no, csl],
                        op0=ALU.add, op1=ALU.add)
            proj_fm(ao_sb, wo_d[l], o_evict)

            h2_sb = layernorm(params.get(("ln2_g", l)), params.get(("ln2_b", l)))

            # MLP per token chunk: hidden chunk lives in SBUF, weights restream
            b1_t = params.get(("b1", l))
            b2_t = params.get(("b2", l))
            for c in range(NCH):
                csl = bass.ts(c, CH)
                hid_sb = hidpool.tile([P, FO, CH], BF16, name="hid", tag="hid")
                fblk = min(4, FO)
                for fb in range(FO // fblk):
                    w1c = load_strip(w1_d[l], 0, E, fb * fblk * P, fblk * P, "w1c")
                    for ni in range(fblk):
                        fo = fb * fblk + ni
                        pst = psum(P, CH, "pss")
                        for ko in range(EO):
                            nc.tensor.matmul(
                                pst, w1c[:, ko, ni * P:(ni + 1) * P],
                                h2_sb[:, ko, csl],
                                start=(ko == 0), stop=(ko == EO - 1))
                        nc.scalar.activation(
                            out=hid_sb[:, fo, :], in_=pst,
                            func=(AF.Tanh if cfg.get("act") == "tanh" else AF.Gelu),
                            bias=(zero_col if b1_t is None
                                  else b1_t[:, fo:fo + 1]))
                for no in range(EO):
                    w2c = load_strip(w2_d[l], 0, FF, no * P, P, "w2c")
                    pst = psum(P, CH, "pss")
                    for ko in range(FO):
                        nc.tensor.matmul(
                            pst, w2c[:, ko, :], hid_sb[:, ko, :],
                            start=(ko == 0), stop=(ko == FO - 1))
                    if b2_t is None:
                        nc.vector.tensor_add(out=x_sb[:, no, csl], in0=pst,
                                             in1=x_sb[:, no, csl])
                    else:
                        nc.vector.scalar_tensor_tensor(
                            out=x_sb[:, no, csl], in0=pst,
                            scalar=b2_t[:, no:no + 1], in1=x_sb[:, no, csl],
                            op0=ALU.add, op1=ALU.add)

        for eo in range(EO):
            nc.sync.dma_start(out=out_d[eo * P:(eo + 1) * P, :], in_=x_sb[:, eo, :])

    nc.finalize()
    return nc


def host_prep(inputs, cfg):
    BL, T, E, H, FF, L = cfg["BL"], cfg["T"], cfg["E"], cfg["H"], cfg["FF"], cfg["L"]
    HD = E // H
    bf = ml_dtypes.bfloat16
    f32 = np.float32
    inp = {k: np.asarray(v, dtype=np.float32) for k, v in inputs.items()}

    shared = {
        "wq": (inp["wq"] * (HD ** -0.5)).astype(bf),
        "wk": inp["wk"].astype(bf),
        "wv": inp["wv"].astype(bf),
        "wo": inp["wo"].astype(bf),
        "w1": inp["w1"].astype(bf),
        "w2": inp["w2"].astype(bf),
    }
    coords = np.arange(T)
    rel = (coords[:, None] - coords[None, :] + (T - 1)).astype(np.float64)
    bias_all = np.empty((L, H, T, T), dtype=bf)
    for l in range(L):
        off = np.tanh(np.float64(inp["offset"][l, 0])) * MAX_OFFSET
        adj = np.clip(rel + off, 0.0, 2.0 * T - 2.0)
        lo = np.floor(adj).astype(np.int64)
        hi = np.ceil(adj).astype(np.int64)
        w = (adj - lo)[..., None].astype(f32)
        tab = inp["bias_table"][l]
        bm = tab[lo] * (1.0 - w) + tab[hi] * w
        bias_all[l] = bm.transpose(2, 0, 1).astype(bf)
    shared["biasmat"] = bias_all

    flags = set()
    for nm, arr in [("bq", (inp["bq"] * (HD ** -0.5)).astype(f32)),
                    ("bk", inp["bk"]), ("bv", inp["bv"]), ("bo", inp["bo"]),
                    ("b1", inp["b1"]), ("b2", inp["b2"])]:
        if np.any(arr):
            flags.add(nm)
            shared[nm] = np.ascontiguousarray(arr, dtype=f32)
    for pre in ("ln1", "ln2"):
        if np.any(inp[f"{pre}_g"] != 1.0) or np.any(inp[f"{pre}_b"]):
            flags.add(pre)
            shared[f"{pre}_g"] = inp[f"{pre}_g"].astype(f32)
            shared[f"{pre}_b"] = inp[f"{pre}_b"].astype(f32)

    per_core_x = []
    for c in range(N_CORES):
        xs = inp["x"][c * BL:(c + 1) * BL]
        per_core_x.append(np.ascontiguousarray(
            xs.transpose(2, 0, 1).reshape(E, BL * T)))
    return shared, per_core_x, frozenset(flags)


_CACHE = {}


def kernel(**inputs) -> np.ndarray:
    cfg = FULL
    BL, T, E = cfg["BL"], cfg["T"], cfg["E"]
    shared, per_core_x, flags = host_prep(inputs, cfg)
    key = ("full", flags)
    if key not in _CACHE:
        _CACHE[key] = build_nc(cfg, flags)
    nc = _CACHE[key]
    in_maps = [{"x_fm": per_core_x[c], **shared} for c in range(N_CORES)]
    res = run_bass_kernel_spmd(nc, in_maps, core_ids=list(range(N_CORES)))
    out = np.empty((N_CORES * BL, T, E), np.float32)
    for c in range(N_CORES):
        ofm = res.results[c]["out_fm"]
        out[c * BL:(c + 1) * BL] = ofm.reshape(E, BL, T).transpose(1, 2, 0)
    return out
